# revision 4
# baseline (speedup 1.0000x reference)
"""VQ codebook kernel v3 (baseline restore) for 8 TRN2 NeuronCores."""

import numpy as np
import ml_dtypes

import concourse.bacc as bacc
import concourse.bass as bass
import concourse.mybir as mybir
from concourse.bass import IndirectOffsetOnAxis
from concourse.bass_utils import run_bass_kernel_spmd
from concourse.tile import TileContext

DIM = 256
K = 8192
B = 8
T = 4096
N_CORES = 8
P = 128
NQ = 4
QK = K // NQ
GW = DIM + 1          # 257 floats per table row
F32 = mybir.dt.float32
F16 = mybir.dt.float16
BF16 = mybir.dt.bfloat16
I32 = mybir.dt.int32
U32 = mybir.dt.uint32
BF = ml_dtypes.bfloat16


def build_nc(t_local: int = T) -> bass.Bass:
    assert t_local % P == 0
    n_tt = t_local // P

    nc = bacc.Bacc("TRN2", target_bir_lowering=False, debug=False)
    xTh_d = nc.declare_dram_parameter("xTh", [DIM, t_local], BF16, isOutput=False)
    x2_d = nc.declare_dram_parameter("x2", [t_local, DIM], F32, isOutput=False)
    ebT_d = nc.declare_dram_parameter("ebT", [DIM, K], BF16, isOutput=False)
    esq2_d = nc.declare_dram_parameter("esq2", [2, K], BF16, isOutput=False)
    tab_d = nc.declare_dram_parameter("tab", [K, GW], F32, isOutput=False)
    out_d = nc.declare_dram_parameter("out", [t_local, DIM], F32, isOutput=True)

    with TileContext(nc) as tc:
        with (
            tc.tile_pool(name="persist", bufs=1) as pp,
            tc.tile_pool(name="psum", bufs=2, space="PSUM") as psum_pool,
            tc.tile_pool(name="xload", bufs=4) as xload,
            tc.tile_pool(name="scores", bufs=2) as scp,
            tc.tile_pool(name="pool", bufs=2) as plp,
            tc.tile_pool(name="gat", bufs=3) as gat,
            tc.tile_pool(name="ms", bufs=2) as msp,
            tc.tile_pool(name="outp", bufs=3) as outp,
            tc.tile_pool(name="small", bufs=4) as small,
        ):
            ebT = pp.tile([P, 2, K], BF16)
            nc.sync.dma_start(
                out=ebT[:], in_=ebT_d[:].rearrange("(a b) k -> b a k", a=2)
            )
            esq2 = pp.tile([2, K], BF16)
            nc.sync.dma_start(out=esq2[:], in_=esq2_d[:])
            ones2 = pp.tile([2, P], BF16)
            nc.vector.memset(ones2[:], 1.0)

            def tile_body(ti):
                tsl = slice(ti * P, (ti + 1) * P)
                xThr = xload.tile([P, 2, P], BF16, tag="xThr")
                nc.sync.dma_start(
                    out=xThr[:],
                    in_=xTh_d[:, tsl].rearrange("(a b) t -> b a t", a=2),
                )
                x2row = xload.tile([P, DIM], F32, tag="x2row")
                nc.sync.dma_start(out=x2row[:], in_=x2_d[tsl, :])

                scores = scp.tile([P, K], F16)
                for q in range(NQ):
                    ps = psum_pool.tile([P, QK], F32)
                    base = q * QK
                    for c in range(2):
                        for b in range(4):
                            ksl = slice(base + b * 512, base + (b + 1) * 512)
                            nc.tensor.matmul(
                                ps[:, b * 512:(b + 1) * 512],
                                lhsT=xThr[:, c, :], rhs=ebT[:, c, ksl],
                                start=(c == 0), stop=False,
                            )
                    for b in range(4):
                        ksl = slice(base + b * 512, base + (b + 1) * 512)
                        nc.tensor.matmul(
                            ps[:, b * 512:(b + 1) * 512],
                            lhsT=ones2[:], rhs=esq2[:, ksl],
                            start=False, stop=True,
                        )
                    nc.scalar.copy(out=scores[:, base:base + QK], in_=ps[:])

                pooled = plp.tile([P, K // 2], F16)
                nc.vector.tensor_tensor(
                    out=pooled[:], in0=scores[:, 0:K // 2],
                    in1=scores[:, K // 2:K], op=mybir.AluOpType.max,
                )
                mx = small.tile([P, 8], F16, tag="mx")
                nc.vector.max(out=mx[:], in_=pooled[:])
                fi = small.tile([P, 8], U32, tag="fi")
                nc.vector.max_index(out=fi[:], in_max=mx[:], in_values=pooled[:])

                gf2 = small.tile([P, 2], F32, tag="gf2")
                nc.vector.tensor_copy(out=gf2[:], in_=fi[:, 0:2])
                ck4 = small.tile([P, 4], F32, tag="ck4")
                nc.vector.tensor_copy(out=ck4[:, 0:3:2], in_=gf2[:])
                nc.vector.tensor_scalar(
                    out=ck4[:, 1:4:2], in0=gf2[:], scalar1=float(K // 2),
                    scalar2=None, op0=mybir.AluOpType.add,
                )
                idx4 = small.tile([P, 4], I32, tag="idx4")
                nc.vector.tensor_copy(out=idx4[:], in_=ck4[:])

                q4 = []
                for s in range(4):
                    qs = gat.tile([P, GW], F32, tag=f"q{s}", name=f"q{s}")
                    nc.gpsimd.indirect_dma_start(
                        out=qs[:], out_offset=None, in_=tab_d[:],
                        in_offset=IndirectOffsetOnAxis(
                            ap=idx4[:, s:s + 1], axis=0),
                    )
                    q4.append(qs)

                ms = msp.tile([P, 4, DIM], F32, tag="ms")
                for s in range(2):
                    nc.vector.tensor_tensor(
                        out=ms[:, s, :], in0=q4[s][:, 0:DIM], in1=x2row[:],
                        op=mybir.AluOpType.mult,
                    )
                for s in range(2, 4):
                    nc.gpsimd.tensor_tensor(
                        out=ms[:, s, :], in0=q4[s][:, 0:DIM], in1=x2row[:],
                        op=mybir.AluOpType.mult,
                    )
                dots4 = small.tile([P, 4], F32, tag="dots4")
                for s in range(2):
                    scr = msp.tile([P, DIM], F32, tag=f"scr{s}")
                    nc.scalar.activation(
                        out=scr[:], in_=ms[:, s, :],
                        func=mybir.ActivationFunctionType.Copy,
                        accum_out=dots4[:, s:s + 1],
                    )
                nc.vector.tensor_reduce(
                    out=dots4[:, 2:4], in_=ms[:, 2:4, :],
                    axis=mybir.AxisListType.X, op=mybir.AluOpType.add,
                )
                sc4 = small.tile([P, 4], F32, tag="sc4")
                for s in range(4):
                    nc.vector.tensor_tensor(
                        out=sc4[:, s:s + 1], in0=dots4[:, s:s + 1],
                        in1=q4[s][:, DIM:DIM + 1],
                        op=mybir.AluOpType.subtract,
                    )
                m1 = small.tile([P, 1], F32, tag="m1")
                nc.vector.reduce_max(
                    out=m1[:], in_=sc4[:], axis=mybir.AxisListType.X
                )
                mask4 = small.tile([P, 4], I32, tag="mask4")
                nc.vector.tensor_scalar(
                    out=mask4[:], in0=sc4[:], scalar1=m1[:, 0:1], scalar2=None,
                    op0=mybir.AluOpType.is_ge,
                )
                out_t = outp.tile([P, DIM], F32)
                nc.vector.tensor_copy(out=out_t[:], in_=q4[0][:, 0:DIM])
                for s in range(1, 4):
                    nc.vector.copy_predicated(
                        out=out_t[:],
                        mask=mask4[:, s:s + 1].to_broadcast([P, DIM]),
                        data=q4[s][:, 0:DIM],
                    )
                nc.gpsimd.dma_start(out=out_d[tsl, :], in_=out_t[:])

            for ti in range(n_tt):
                tile_body(ti)

    nc.compile()
    return nc


def prep_core_inputs(x_i: np.ndarray, shared: dict) -> dict:
    x2 = (2.0 * x_i).astype(np.float32)
    xTh = np.ascontiguousarray(x2.astype(BF).T)
    return {
        "xTh": xTh,
        "x2": np.ascontiguousarray(x2),
        **shared,
    }


def prep_shared(embed: np.ndarray) -> dict:
    perm = np.concatenate([np.arange(0, K, 2), np.arange(1, K, 2)])
    embed = embed[perm]
    esq = (embed.astype(np.float64) ** 2).sum(1).astype(np.float32)
    neg = -esq
    hi = neg.astype(BF)
    lo = (neg - hi.astype(np.float32)).astype(BF)
    esq2 = np.stack([hi, lo], axis=0)
    ebT = np.ascontiguousarray(embed.astype(BF).T)
    tab = np.concatenate([embed, esq[:, None]], axis=1).astype(np.float32)
    return {"ebT": ebT, "esq2": esq2, "tab": np.ascontiguousarray(tab)}


def kernel(x: np.ndarray, embed: np.ndarray) -> np.ndarray:
    x = np.ascontiguousarray(x, dtype=np.float32)
    embed = np.ascontiguousarray(embed, dtype=np.float32)
    assert x.shape == (B, T, DIM), x.shape
    assert embed.shape == (K, DIM), embed.shape

    nc = build_nc(T)
    shared = prep_shared(embed)
    in_maps = [prep_core_inputs(x[i], shared) for i in range(N_CORES)]
    res = run_bass_kernel_spmd(nc, in_maps, core_ids=list(range(N_CORES)))
    out = np.stack([res.results[i]["out"] for i in range(N_CORES)], axis=0)
    return out.astype(np.float32)


# revision 6
# speedup vs baseline: 1.2597x; 1.2597x over previous
"""VQ codebook kernel v5 for 8 TRN2 NeuronCores.

Data-parallel over batch: each core handles one batch element (4096 tokens).

Per 128-token tile:
  - PE (fp16): coarse scores s[t,k] = fp16(2x_t).fp16(e_k) - |e_k|^2 into
    PSUM quarters; the -|e|^2 rows (fp16 hi+lo) are added by four K=2
    matmuls per quarter packed into distinct 32-row groups via
    tile_position so they run concurrently (~1 matmul slot instead of 4).
  - ACT evicts each quarter to fp16 SBUF scores.
  - DVE tree: pooled pair halves (split so each starts as soon as its two
    quarters are evicted) -> pooled2[2048] (4-code slots) -> L4[1024]
    (8-code groups); max8 over L4; find_index8 over pooled2 locates the
    best 4-code slot of the top-2 groups (duplicate in_max values return
    distinct positions, covering f16 ties).
  - gather: tab4 rows hold all 4 codes of a slot (+ their -|e|^2); two
    indirect gathers fetch 8 candidate codes (descriptor-count-bound, so
    the wide rows cost the same as narrow ones).
  - Rescore: per gathered tile one big multiply (x2 broadcast over the 4
    code chunks) + one segmented tensor_reduce + one FD4 add of the
    gathered -|e|^2 tail; tile g0 on DVE, g1 on gpsimd. All exact f32.
  - Winner: integer id math (slot id + 2048*c), is_ge mask, masked-max
    reduce -> id; indirect gather from the f32 codebook + store.
"""

import numpy as np
import ml_dtypes

import concourse.bacc as bacc
import concourse.bass as bass
import concourse.mybir as mybir
from concourse.bass import IndirectOffsetOnAxis
from concourse.bass_utils import run_bass_kernel_spmd
from concourse.tile import TileContext

DIM = 256
K = 8192
B = 8
T = 4096
N_CORES = 8
P = 128
NQ = 4
QK = K // NQ          # 2048
HK = K // 2           # 4096
GW4 = 4 * (DIM + 1)   # 1028 floats per tab4 row
F32 = mybir.dt.float32
F16 = mybir.dt.float16
I32 = mybir.dt.int32
U32 = mybir.dt.uint32
NF16 = np.float16
N_GP_STT = 3          # candidates rescored on gpsimd (rest on DVE)


def build_nc(t_local: int = T) -> bass.Bass:
    assert t_local % P == 0
    n_tt = t_local // P

    nc = bacc.Bacc("TRN2", target_bir_lowering=False, debug=False)
    xTh_d = nc.declare_dram_parameter("xTh", [DIM, t_local], F16, isOutput=False)
    x2_d = nc.declare_dram_parameter("x2", [t_local, DIM], F32, isOutput=False)
    ebT_d = nc.declare_dram_parameter("ebT", [DIM, K], F16, isOutput=False)
    esq8_d = nc.declare_dram_parameter("esq8", [4, 2, NQ, 512], F16, isOutput=False)
    tab4_d = nc.declare_dram_parameter("tab4", [QK, GW4], F32, isOutput=False)
    tabW_d = nc.declare_dram_parameter("tabW", [K, DIM], F32, isOutput=False)
    out_d = nc.declare_dram_parameter("out", [t_local, DIM], F32, isOutput=True)

    with TileContext(nc) as tc:
        with (
            tc.tile_pool(name="persist", bufs=1) as pp,
            tc.tile_pool(name="psum", bufs=2, space="PSUM") as psum_pool,
            tc.tile_pool(name="xload", bufs=4) as xload,
            tc.tile_pool(name="scores", bufs=3) as scp,
            tc.tile_pool(name="pool", bufs=2) as plp,
            tc.tile_pool(name="gat", bufs=3) as gat,
            tc.tile_pool(name="ms", bufs=4) as msp,
            tc.tile_pool(name="small", bufs=4) as small,
        ):
            ebT = pp.tile([P, 2, K], F16)
            nc.sync.dma_start(
                out=ebT[:], in_=ebT_d[:].rearrange("(a b) k -> b a k", a=2)
            )
            # -|e|^2 fp16 hi+lo rows replicated at partition bases 0/32/64/96
            # so the four aug matmuls of a quarter can row-tile concurrently
            esq8 = pp.tile([P, NQ, 512], F16)
            for b in range(4):
                nc.sync.dma_start(
                    out=esq8[32 * b:32 * b + 2, :, :], in_=esq8_d[b]
                )
            ones8 = pp.tile([P, P], F16)
            nc.vector.memset(ones8[:], 1.0)
            # candidate id offsets: [0,2048,4096,6144] per slot, both slots
            const8 = pp.tile([P, 8], I32)
            nc.gpsimd.iota(const8[:], pattern=[[0, 2], [2048, 4]], base=0,
                           channel_multiplier=0)

            def tile_body(ti):
                tsl = slice(ti * P, (ti + 1) * P)
                xThr = xload.tile([P, 2, P], F16, tag="xThr")
                nc.sync.dma_start(
                    out=xThr[:],
                    in_=xTh_d[:, tsl].rearrange("(a b) t -> b a t", a=2),
                )
                x2row = xload.tile([P, DIM], F32, tag="x2row")
                nc.sync.dma_start(out=x2row[:], in_=x2_d[tsl, :])

                scores = scp.tile([P, K], F16)
                for q in range(NQ):
                    ps = psum_pool.tile([P, QK], F32)
                    base = q * QK
                    for c in range(2):
                        for b in range(4):
                            ksl = slice(base + b * 512, base + (b + 1) * 512)
                            nc.tensor.matmul(
                                ps[:, b * 512:(b + 1) * 512],
                                lhsT=xThr[:, c, :], rhs=ebT[:, c, ksl],
                                start=(c == 0), stop=False,
                            )
                    for b in range(4):
                        nc.tensor.matmul(
                            ps[:, b * 512:(b + 1) * 512],
                            lhsT=ones8[32 * b:32 * b + 2, :],
                            rhs=esq8[32 * b:32 * b + 2, q, :],
                            start=False, stop=True,
                            tile_position=(32 * b, 0),
                        )
                    nc.scalar.copy(out=scores[:, base:base + QK], in_=ps[:])

                # tree: pooled pair halves at [0:2048],[2048:4096] (pairs
                # (j, j+4096)); pooled2 (4-code slots) at [4096:6144];
                # L4 (8-code groups) at [6144:7168]
                pooled = plp.tile([P, 7168], F16)
                nc.vector.tensor_tensor(
                    out=pooled[:, 0:2048], in0=scores[:, 0:2048],
                    in1=scores[:, HK:HK + 2048], op=mybir.AluOpType.max,
                )
                nc.vector.tensor_tensor(
                    out=pooled[:, 2048:HK], in0=scores[:, 2048:HK],
                    in1=scores[:, HK + 2048:K], op=mybir.AluOpType.max,
                )
                nc.vector.tensor_tensor(
                    out=pooled[:, HK:HK + 2048], in0=pooled[:, 0:2048],
                    in1=pooled[:, 2048:HK], op=mybir.AluOpType.max,
                )
                nc.vector.tensor_tensor(
                    out=pooled[:, HK + 2048:HK + 3072],
                    in0=pooled[:, HK:HK + 1024],
                    in1=pooled[:, HK + 1024:HK + 2048],
                    op=mybir.AluOpType.max,
                )
                mx = small.tile([P, 8], F16, tag="mx")
                nc.vector.max(out=mx[:], in_=pooled[:, HK + 2048:HK + 3072])
                fi = small.tile([P, 8], U32, tag="fi")
                nc.vector.max_index(
                    out=fi[:], in_max=mx[:], in_values=pooled[:, HK:HK + 2048]
                )

                idx2 = small.tile([P, 2], I32, tag="idx2")
                nc.vector.tensor_copy(out=idx2[:], in_=fi[:, 0:2])

                # two slot-gathers: row = [e_a, -esq_a, e_b, -esq_b, e_c,
                # -esq_c, e_d, -esq_d] covering codes {j, j+2048, j+4096,
                # j+6144}
                q2 = []
                for s in range(2):
                    qs = gat.tile([P, GW4], F32, tag=f"q{s}", name=f"q{s}")
                    nc.gpsimd.indirect_dma_start(
                        out=qs[:], out_offset=None, in_=tab4_d[:],
                        in_offset=IndirectOffsetOnAxis(
                            ap=idx2[:, s:s + 1], axis=0),
                    )
                    q2.append(qs)

                # exact rescore of 8 candidates: per gathered tile, one big
                # multiply (x2 broadcast across the 4 code chunks) + one
                # segmented tensor_reduce + one contiguous FD4 add of the
                # gathered -|e|^2 tail. g0 on DVE, g1 on gpsimd.
                sc8 = small.tile([P, 8], F32, tag="sc8")
                dots8 = small.tile([P, 8], F32, tag="dots8")
                x2b = x2row[:].rearrange("p (o d) -> p o d", o=1).to_broadcast([P, 4, DIM])
                for g in range(2):
                    eng = nc.vector if g == 0 else nc.gpsimd
                    ms = msp.tile([P, 4, DIM], F32, tag=f"ms{g}")
                    eng.tensor_tensor(
                        out=ms[:],
                        in0=q2[g][:, 0:4 * DIM].rearrange(
                            "p (c d) -> p c d", d=DIM),
                        in1=x2b, op=mybir.AluOpType.mult,
                    )
                    nc.vector.tensor_reduce(
                        out=dots8[:, g * 4:(g + 1) * 4], in_=ms[:],
                        axis=mybir.AxisListType.X, op=mybir.AluOpType.add,
                    )
                    nc.vector.tensor_tensor(
                        out=sc8[:, g * 4:(g + 1) * 4],
                        in0=dots8[:, g * 4:(g + 1) * 4],
                        in1=q2[g][:, 4 * DIM:4 * DIM + 4],
                        op=mybir.AluOpType.add,
                    )

                # candidate ids: ids8[s] = slot_g + 2048*c  (int path)
                ids8 = small.tile([P, 8], I32, tag="ids8")
                nc.vector.tensor_tensor(
                    out=ids8[:].rearrange("p (g c) -> p g c", g=2),
                    in0=idx2[:].rearrange("p (g c) -> p g c", c=1).to_broadcast([P, 2, 4]),
                    in1=const8[:].rearrange("p (g c) -> p g c", g=2),
                    op=mybir.AluOpType.add,
                )
                m1 = small.tile([P, 1], F32, tag="m1")
                nc.vector.reduce_max(
                    out=m1[:], in_=sc8[:], axis=mybir.AxisListType.X
                )
                mask8 = small.tile([P, 8], I32, tag="mask8")
                nc.vector.tensor_scalar(
                    out=mask8[:], in0=sc8[:], scalar1=m1[:, 0:1], scalar2=None,
                    op0=mybir.AluOpType.is_ge,
                )
                mid8 = small.tile([P, 8], I32, tag="mid8")
                nc.vector.tensor_tensor(
                    out=mid8[:], in0=mask8[:], in1=ids8[:],
                    op=mybir.AluOpType.mult,
                )
                wid = small.tile([P, 1], I32, tag="wid")
                nc.vector.reduce_max(
                    out=wid[:], in_=mid8[:], axis=mybir.AxisListType.X
                )

                outw = gat.tile([P, DIM], F32, tag="outw", name="outw")
                nc.gpsimd.indirect_dma_start(
                    out=outw[:], out_offset=None, in_=tabW_d[:],
                    in_offset=IndirectOffsetOnAxis(ap=wid[:, 0:1], axis=0),
                )
                nc.gpsimd.dma_start(out=out_d[tsl, :], in_=outw[:])

            for ti in range(n_tt):
                tile_body(ti)

    nc.compile()
    return nc


def prep_core_inputs(x_i: np.ndarray, shared: dict) -> dict:
    x2 = (2.0 * x_i).astype(np.float32)
    xTh = np.ascontiguousarray(x2.astype(NF16).T)
    return {
        "xTh": xTh,
        "x2": np.ascontiguousarray(x2),
        **shared,
    }


def prep_shared(embed: np.ndarray) -> dict:
    # permute codebook (evens then odds): pool pair j = permuted rows
    # (j, j+K/2); pooled2 slot j covers {j, j+2048, j+4096, j+6144}
    perm = np.concatenate([np.arange(0, K, 2), np.arange(1, K, 2)])
    eP = embed[perm]
    esq = (eP.astype(np.float64) ** 2).sum(1).astype(np.float32)
    neg = -esq
    hi = neg.astype(NF16)
    lo = (neg - hi.astype(np.float32)).astype(NF16)
    # esq8[b, r, q, j] = (hi,lo)[r][q*2048 + b*512 + j]
    esqr = np.stack([hi, lo], axis=0).reshape(2, NQ, 4, 512)  # [r, q, b, j]
    esq8 = np.ascontiguousarray(esqr.transpose(2, 0, 1, 3))   # [b, r, q, j]
    ebT = np.ascontiguousarray(eP.astype(NF16).T)
    # tab4 row j = [e_j, e_{j+2048}, e_{j+4096}, e_{j+6144} (1024),
    #               -esq_j, -esq_{j+2048}, -esq_{j+4096}, -esq_{j+6144}]
    blocks = [eP[c * QK:(c + 1) * QK] for c in range(4)]
    blocks += [neg.reshape(4, QK).T]
    tab4 = np.concatenate(blocks, axis=1).astype(np.float32)
    return {
        "ebT": ebT,
        "esq8": esq8,
        "tab4": np.ascontiguousarray(tab4),
        "tabW": np.ascontiguousarray(eP.astype(np.float32)),
    }


def kernel(x: np.ndarray, embed: np.ndarray) -> np.ndarray:
    x = np.ascontiguousarray(x, dtype=np.float32)
    embed = np.ascontiguousarray(embed, dtype=np.float32)
    assert x.shape == (B, T, DIM), x.shape
    assert embed.shape == (K, DIM), embed.shape

    nc = build_nc(T)
    shared = prep_shared(embed)
    in_maps = [prep_core_inputs(x[i], shared) for i in range(N_CORES)]
    res = run_bass_kernel_spmd(nc, in_maps, core_ids=list(range(N_CORES)))
    out = np.stack([res.results[i]["out"] for i in range(N_CORES)], axis=0)
    return out.astype(np.float32)


if __name__ == "__main__":
    rng = np.random.default_rng(0)
    x = rng.standard_normal((B, T, DIM), dtype=np.float32)
    embed = rng.standard_normal((K, DIM), dtype=np.float32)
    out = kernel(x, embed)
    flat = x.reshape(-1, DIM)
    d = (flat * flat).sum(1)[:, None] - 2.0 * flat @ embed.T + (embed * embed).sum(1)[None, :]
    ref = embed[np.argmin(d, axis=1)].reshape(B, T, DIM)
    err = np.abs(out - ref).max()
    print("max abs err vs numpy ref:", err)


# revision 7
# speedup vs baseline: 1.2638x; 1.0032x over previous
"""VQ codebook kernel v5 for 8 TRN2 NeuronCores.

Data-parallel over batch: each core handles one batch element (4096 tokens).

Per 128-token tile:
  - PE (fp16): coarse scores s[t,k] = fp16(2x_t).fp16(e_k) - |e_k|^2 into
    PSUM quarters; the -|e|^2 rows (fp16 hi+lo) are added by four K=2
    matmuls per quarter packed into distinct 32-row groups via
    tile_position so they run concurrently (~1 matmul slot instead of 4).
  - ACT evicts each quarter to fp16 SBUF scores.
  - DVE tree: pooled pair halves (split so each starts as soon as its two
    quarters are evicted) -> pooled2[2048] (4-code slots) -> L4[1024]
    (8-code groups); max8 over L4; find_index8 over pooled2 locates the
    best 4-code slot of the top-2 groups (duplicate in_max values return
    distinct positions, covering f16 ties).
  - gather: tab4 rows hold all 4 codes of a slot (+ their -|e|^2); two
    indirect gathers fetch 8 candidate codes (descriptor-count-bound, so
    the wide rows cost the same as narrow ones).
  - Rescore: per gathered tile one big multiply (x2 broadcast over the 4
    code chunks) + one segmented tensor_reduce + one FD4 add of the
    gathered -|e|^2 tail; tile g0 on DVE, g1 on gpsimd. All exact f32.
  - Winner: integer id math (slot id + 2048*c), is_ge mask, masked-max
    reduce -> id; indirect gather from the f32 codebook + store.
"""

import numpy as np
import ml_dtypes

import concourse.bacc as bacc
import concourse.bass as bass
import concourse.mybir as mybir
from concourse.bass import IndirectOffsetOnAxis
from concourse.bass_utils import run_bass_kernel_spmd
from concourse.tile import TileContext

DIM = 256
K = 8192
B = 8
T = 4096
N_CORES = 8
P = 128
NQ = 4
QK = K // NQ          # 2048
HK = K // 2           # 4096
GW4 = 4 * (DIM + 1)   # 1028 floats per tab4 row
F32 = mybir.dt.float32
F16 = mybir.dt.float16
I32 = mybir.dt.int32
U32 = mybir.dt.uint32
NF16 = np.float16
N_GP_STT = 3          # candidates rescored on gpsimd (rest on DVE)


def build_nc(t_local: int = T) -> bass.Bass:
    assert t_local % P == 0
    n_tt = t_local // P

    nc = bacc.Bacc("TRN2", target_bir_lowering=False, debug=False)
    xTh_d = nc.declare_dram_parameter("xTh", [DIM, t_local], F16, isOutput=False)
    x2_d = nc.declare_dram_parameter("x2", [t_local, DIM], F32, isOutput=False)
    ebT_d = nc.declare_dram_parameter("ebT", [DIM, K], F16, isOutput=False)
    esq8_d = nc.declare_dram_parameter("esq8", [4, 2, NQ, 512], F16, isOutput=False)
    tab4_d = nc.declare_dram_parameter("tab4", [QK, GW4], F32, isOutput=False)
    tabW_d = nc.declare_dram_parameter("tabW", [K, DIM], F32, isOutput=False)
    out_d = nc.declare_dram_parameter("out", [t_local, DIM], F32, isOutput=True)

    with TileContext(nc) as tc:
        with (
            tc.tile_pool(name="persist", bufs=1) as pp,
            tc.tile_pool(name="psum", bufs=2, space="PSUM") as psum_pool,
            tc.tile_pool(name="xload", bufs=4) as xload,
            tc.tile_pool(name="scores", bufs=3) as scp,
            tc.tile_pool(name="pool", bufs=2) as plp,
            tc.tile_pool(name="gat", bufs=3) as gat,
            tc.tile_pool(name="ms", bufs=4) as msp,
            tc.tile_pool(name="small", bufs=4) as small,
        ):
            ebT = pp.tile([P, 2, K], F16)
            nc.sync.dma_start(
                out=ebT[:], in_=ebT_d[:].rearrange("(a b) k -> b a k", a=2)
            )
            # -|e|^2 fp16 hi+lo rows replicated at partition bases 0/32/64/96
            # so the four aug matmuls of a quarter can row-tile concurrently
            esq8 = pp.tile([P, NQ, 512], F16)
            for b in range(4):
                nc.sync.dma_start(
                    out=esq8[32 * b:32 * b + 2, :, :], in_=esq8_d[b]
                )
            ones8 = pp.tile([P, P], F16)
            nc.vector.memset(ones8[:], 1.0)
            # candidate id offsets: [0,2048,4096,6144] per slot, both slots
            const8 = pp.tile([P, 8], I32)
            nc.gpsimd.iota(const8[:], pattern=[[0, 2], [2048, 4]], base=0,
                           channel_multiplier=0)

            state = {}

            def tile_front(ti):
                tsl = slice(ti * P, (ti + 1) * P)
                xThr = xload.tile([P, 2, P], F16, tag="xThr")
                nc.sync.dma_start(
                    out=xThr[:],
                    in_=xTh_d[:, tsl].rearrange("(a b) t -> b a t", a=2),
                )
                x2row = xload.tile([P, DIM], F32, tag="x2row")
                nc.sync.dma_start(out=x2row[:], in_=x2_d[tsl, :])

                scores = scp.tile([P, K], F16)
                for q in range(NQ):
                    ps = psum_pool.tile([P, QK], F32)
                    base = q * QK
                    for c in range(2):
                        for b in range(4):
                            ksl = slice(base + b * 512, base + (b + 1) * 512)
                            nc.tensor.matmul(
                                ps[:, b * 512:(b + 1) * 512],
                                lhsT=xThr[:, c, :], rhs=ebT[:, c, ksl],
                                start=(c == 0), stop=False,
                            )
                    for b in range(4):
                        nc.tensor.matmul(
                            ps[:, b * 512:(b + 1) * 512],
                            lhsT=ones8[32 * b:32 * b + 2, :],
                            rhs=esq8[32 * b:32 * b + 2, q, :],
                            start=False, stop=True,
                            tile_position=(32 * b, 0),
                        )
                    nc.scalar.copy(out=scores[:, base:base + QK], in_=ps[:])

                # tree: pooled pair halves at [0:2048],[2048:4096] (pairs
                # (j, j+4096)); pooled2 (4-code slots) at [4096:6144];
                # L4 (8-code groups) at [6144:7168]
                pooled = plp.tile([P, 7168], F16)
                nc.vector.tensor_tensor(
                    out=pooled[:, 0:2048], in0=scores[:, 0:2048],
                    in1=scores[:, HK:HK + 2048], op=mybir.AluOpType.max,
                )
                nc.vector.tensor_tensor(
                    out=pooled[:, 2048:HK], in0=scores[:, 2048:HK],
                    in1=scores[:, HK + 2048:K], op=mybir.AluOpType.max,
                )
                nc.vector.tensor_tensor(
                    out=pooled[:, HK:HK + 2048], in0=pooled[:, 0:2048],
                    in1=pooled[:, 2048:HK], op=mybir.AluOpType.max,
                )
                nc.vector.tensor_tensor(
                    out=pooled[:, HK + 2048:HK + 3072],
                    in0=pooled[:, HK:HK + 1024],
                    in1=pooled[:, HK + 1024:HK + 2048],
                    op=mybir.AluOpType.max,
                )
                mx = small.tile([P, 8], F16, tag="mx")
                nc.vector.max(out=mx[:], in_=pooled[:, HK + 2048:HK + 3072])
                fi = small.tile([P, 8], U32, tag="fi")
                nc.vector.max_index(
                    out=fi[:], in_max=mx[:], in_values=pooled[:, HK:HK + 2048]
                )

                idx2 = small.tile([P, 2], I32, tag="idx2")
                nc.vector.tensor_copy(out=idx2[:], in_=fi[:, 0:2])

                # two slot-gathers: row = [e_a, -esq_a, e_b, -esq_b, e_c,
                # -esq_c, e_d, -esq_d] covering codes {j, j+2048, j+4096,
                # j+6144}
                q2 = []
                for s in range(2):
                    qs = gat.tile([P, GW4], F32, tag=f"q{s}", name=f"q{s}")
                    nc.gpsimd.indirect_dma_start(
                        out=qs[:], out_offset=None, in_=tab4_d[:],
                        in_offset=IndirectOffsetOnAxis(
                            ap=idx2[:, s:s + 1], axis=0),
                    )
                    q2.append(qs)
                state[ti] = (q2, idx2, x2row)

            def tile_back(ti):
                tsl = slice(ti * P, (ti + 1) * P)
                q2, idx2, x2row = state.pop(ti)

                # exact rescore of 8 candidates: per gathered tile, one big
                # multiply (x2 broadcast across the 4 code chunks) + one
                # segmented tensor_reduce + one contiguous FD4 add of the
                # gathered -|e|^2 tail. g0 on DVE, g1 on gpsimd.
                sc8 = small.tile([P, 8], F32, tag="sc8")
                dots8 = small.tile([P, 8], F32, tag="dots8")
                x2b = x2row[:].rearrange("p (o d) -> p o d", o=1).to_broadcast([P, 4, DIM])
                for g in range(2):
                    eng = nc.vector if g == 0 else nc.gpsimd
                    ms = msp.tile([P, 4, DIM], F32, tag=f"ms{g}")
                    eng.tensor_tensor(
                        out=ms[:],
                        in0=q2[g][:, 0:4 * DIM].rearrange(
                            "p (c d) -> p c d", d=DIM),
                        in1=x2b, op=mybir.AluOpType.mult,
                    )
                    nc.vector.tensor_reduce(
                        out=dots8[:, g * 4:(g + 1) * 4], in_=ms[:],
                        axis=mybir.AxisListType.X, op=mybir.AluOpType.add,
                    )
                    nc.vector.tensor_tensor(
                        out=sc8[:, g * 4:(g + 1) * 4],
                        in0=dots8[:, g * 4:(g + 1) * 4],
                        in1=q2[g][:, 4 * DIM:4 * DIM + 4],
                        op=mybir.AluOpType.add,
                    )

                # candidate ids: ids8[s] = slot_g + 2048*c  (int path)
                ids8 = small.tile([P, 8], I32, tag="ids8")
                nc.vector.tensor_tensor(
                    out=ids8[:].rearrange("p (g c) -> p g c", g=2),
                    in0=idx2[:].rearrange("p (g c) -> p g c", c=1).to_broadcast([P, 2, 4]),
                    in1=const8[:].rearrange("p (g c) -> p g c", g=2),
                    op=mybir.AluOpType.add,
                )
                m1 = small.tile([P, 1], F32, tag="m1")
                nc.vector.reduce_max(
                    out=m1[:], in_=sc8[:], axis=mybir.AxisListType.X
                )
                mask8 = small.tile([P, 8], I32, tag="mask8")
                nc.vector.tensor_scalar(
                    out=mask8[:], in0=sc8[:], scalar1=m1[:, 0:1], scalar2=None,
                    op0=mybir.AluOpType.is_ge,
                )
                mid8 = small.tile([P, 8], I32, tag="mid8")
                nc.vector.tensor_tensor(
                    out=mid8[:], in0=mask8[:], in1=ids8[:],
                    op=mybir.AluOpType.mult,
                )
                wid = small.tile([P, 1], I32, tag="wid")
                nc.vector.reduce_max(
                    out=wid[:], in_=mid8[:], axis=mybir.AxisListType.X
                )

                outw = gat.tile([P, DIM], F32, tag="outw", name="outw")
                nc.gpsimd.indirect_dma_start(
                    out=outw[:], out_offset=None, in_=tabW_d[:],
                    in_offset=IndirectOffsetOnAxis(ap=wid[:, 0:1], axis=0),
                )
                nc.gpsimd.dma_start(out=out_d[tsl, :], in_=outw[:])

            for ti in range(n_tt):
                tile_front(ti)
                if ti > 0:
                    tile_back(ti - 1)
            tile_back(n_tt - 1)

    nc.compile()
    return nc


def prep_core_inputs(x_i: np.ndarray, shared: dict) -> dict:
    x2 = (2.0 * x_i).astype(np.float32)
    xTh = np.ascontiguousarray(x2.astype(NF16).T)
    return {
        "xTh": xTh,
        "x2": np.ascontiguousarray(x2),
        **shared,
    }


def prep_shared(embed: np.ndarray) -> dict:
    # permute codebook (evens then odds): pool pair j = permuted rows
    # (j, j+K/2); pooled2 slot j covers {j, j+2048, j+4096, j+6144}
    perm = np.concatenate([np.arange(0, K, 2), np.arange(1, K, 2)])
    eP = embed[perm]
    esq = (eP.astype(np.float64) ** 2).sum(1).astype(np.float32)
    neg = -esq
    hi = neg.astype(NF16)
    lo = (neg - hi.astype(np.float32)).astype(NF16)
    # esq8[b, r, q, j] = (hi,lo)[r][q*2048 + b*512 + j]
    esqr = np.stack([hi, lo], axis=0).reshape(2, NQ, 4, 512)  # [r, q, b, j]
    esq8 = np.ascontiguousarray(esqr.transpose(2, 0, 1, 3))   # [b, r, q, j]
    ebT = np.ascontiguousarray(eP.astype(NF16).T)
    # tab4 row j = [e_j, e_{j+2048}, e_{j+4096}, e_{j+6144} (1024),
    #               -esq_j, -esq_{j+2048}, -esq_{j+4096}, -esq_{j+6144}]
    blocks = [eP[c * QK:(c + 1) * QK] for c in range(4)]
    blocks += [neg.reshape(4, QK).T]
    tab4 = np.concatenate(blocks, axis=1).astype(np.float32)
    return {
        "ebT": ebT,
        "esq8": esq8,
        "tab4": np.ascontiguousarray(tab4),
        "tabW": np.ascontiguousarray(eP.astype(np.float32)),
    }


def kernel(x: np.ndarray, embed: np.ndarray) -> np.ndarray:
    x = np.ascontiguousarray(x, dtype=np.float32)
    embed = np.ascontiguousarray(embed, dtype=np.float32)
    assert x.shape == (B, T, DIM), x.shape
    assert embed.shape == (K, DIM), embed.shape

    nc = build_nc(T)
    shared = prep_shared(embed)
    in_maps = [prep_core_inputs(x[i], shared) for i in range(N_CORES)]
    res = run_bass_kernel_spmd(nc, in_maps, core_ids=list(range(N_CORES)))
    out = np.stack([res.results[i]["out"] for i in range(N_CORES)], axis=0)
    return out.astype(np.float32)


if __name__ == "__main__":
    rng = np.random.default_rng(0)
    x = rng.standard_normal((B, T, DIM), dtype=np.float32)
    embed = rng.standard_normal((K, DIM), dtype=np.float32)
    out = kernel(x, embed)
    flat = x.reshape(-1, DIM)
    d = (flat * flat).sum(1)[:, None] - 2.0 * flat @ embed.T + (embed * embed).sum(1)[None, :]
    ref = embed[np.argmin(d, axis=1)].reshape(B, T, DIM)
    err = np.abs(out - ref).max()
    print("max abs err vs numpy ref:", err)


# revision 8
# speedup vs baseline: 1.2806x; 1.0133x over previous
"""VQ codebook kernel v5 for 8 TRN2 NeuronCores.

Data-parallel over batch: each core handles one batch element (4096 tokens).

Per 128-token tile:
  - PE (fp16): coarse scores s[t,k] = fp16(2x_t).fp16(e_k) - |e_k|^2 into
    PSUM quarters; the -|e|^2 rows (fp16 hi+lo) are added by four K=2
    matmuls per quarter packed into distinct 32-row groups via
    tile_position so they run concurrently (~1 matmul slot instead of 4).
  - ACT evicts each quarter to fp16 SBUF scores.
  - DVE tree: pooled pair halves (split so each starts as soon as its two
    quarters are evicted) -> pooled2[2048] (4-code slots) -> L4[1024]
    (8-code groups); max8 over L4; find_index8 over pooled2 locates the
    best 4-code slot of the top-2 groups (duplicate in_max values return
    distinct positions, covering f16 ties).
  - gather: tab4 rows hold all 4 codes of a slot (+ their -|e|^2); two
    indirect gathers fetch 8 candidate codes (descriptor-count-bound, so
    the wide rows cost the same as narrow ones).
  - Rescore: per gathered tile one big multiply (x2 broadcast over the 4
    code chunks) + one segmented tensor_reduce + one FD4 add of the
    gathered -|e|^2 tail; tile g0 on DVE, g1 on gpsimd. All exact f32.
  - Winner: integer id math (slot id + 2048*c), is_ge mask, masked-max
    reduce -> id; indirect gather from the f32 codebook + store.
"""

import numpy as np
import ml_dtypes

import concourse.bacc as bacc
import concourse.bass as bass
import concourse.mybir as mybir
from concourse.bass import IndirectOffsetOnAxis
from concourse.bass_utils import run_bass_kernel_spmd
from concourse.tile import TileContext

DIM = 256
K = 8192
B = 8
T = 4096
N_CORES = 8
P = 128
NQ = 4
QK = K // NQ          # 2048
HK = K // 2           # 4096
GW4 = 4 * (DIM + 1)   # 1028 floats per tab4 row
F32 = mybir.dt.float32
F16 = mybir.dt.float16
I32 = mybir.dt.int32
U32 = mybir.dt.uint32
NF16 = np.float16
N_GP_STT = 3          # candidates rescored on gpsimd (rest on DVE)


def build_nc(t_local: int = T) -> bass.Bass:
    assert t_local % P == 0
    n_tt = t_local // P

    nc = bacc.Bacc("TRN2", target_bir_lowering=False, debug=False)
    xTh_d = nc.declare_dram_parameter("xTh", [DIM, t_local], F16, isOutput=False)
    x2_d = nc.declare_dram_parameter("x2", [t_local, DIM + 1], F32, isOutput=False)
    ebT_d = nc.declare_dram_parameter("ebT", [DIM, K], F16, isOutput=False)
    esq8_d = nc.declare_dram_parameter("esq8", [4, 2, NQ, 512], F16, isOutput=False)
    tab4_d = nc.declare_dram_parameter("tab4", [QK, GW4], F32, isOutput=False)
    tabW_d = nc.declare_dram_parameter("tabW", [K, DIM], F32, isOutput=False)
    out_d = nc.declare_dram_parameter("out", [t_local, DIM], F32, isOutput=True)

    with TileContext(nc) as tc:
        with (
            tc.tile_pool(name="persist", bufs=1) as pp,
            tc.tile_pool(name="psum", bufs=4, space="PSUM") as psum_pool,
            tc.tile_pool(name="xload", bufs=4) as xload,
            tc.tile_pool(name="scores", bufs=3) as scp,
            tc.tile_pool(name="pool", bufs=2) as plp,
            tc.tile_pool(name="gat", bufs=3) as gat,
            tc.tile_pool(name="ms", bufs=4) as msp,
            tc.tile_pool(name="small", bufs=4) as small,
        ):
            ebT = pp.tile([P, 2, K], F16)
            nc.sync.dma_start(
                out=ebT[:], in_=ebT_d[:].rearrange("(a b) k -> b a k", a=2)
            )
            # -|e|^2 fp16 hi+lo rows replicated at partition bases 0/32/64/96
            # so the four aug matmuls of a quarter can row-tile concurrently
            esq8 = pp.tile([P, NQ, 512], F16)
            for b in range(4):
                nc.sync.dma_start(
                    out=esq8[32 * b:32 * b + 2, :, :], in_=esq8_d[b]
                )
            ones8 = pp.tile([P, P], F16)
            nc.vector.memset(ones8[:], 1.0)
            # candidate id offsets: [0,2048,4096,6144] per slot, both slots
            const8 = pp.tile([P, 8], I32)
            nc.gpsimd.iota(const8[:], pattern=[[0, 2], [2048, 4]], base=0,
                           channel_multiplier=0)

            state = {}

            def tile_front(ti):
                tsl = slice(ti * P, (ti + 1) * P)
                xThr = xload.tile([P, 2, P], F16, tag="xThr")
                nc.sync.dma_start(
                    out=xThr[:],
                    in_=xTh_d[:, tsl].rearrange("(a b) t -> b a t", a=2),
                )
                x2row = xload.tile([P, DIM + 1], F32, tag="x2row")
                nc.sync.dma_start(out=x2row[:], in_=x2_d[tsl, :])

                scq = [scp.tile([P, QK], F16, tag=f"scq{q}", name=f"scq{q}")
                       for q in range(NQ)]
                for e in range(8):
                    q, h = divmod(e, 2)
                    ps = psum_pool.tile([P, 1024], F32)
                    for c in range(2):
                        for bb in range(2):
                            b = 2 * h + bb
                            ksl = slice(q * QK + b * 512, q * QK + (b + 1) * 512)
                            nc.tensor.matmul(
                                ps[:, bb * 512:(bb + 1) * 512],
                                lhsT=xThr[:, c, :], rhs=ebT[:, c, ksl],
                                start=(c == 0), stop=False,
                            )
                    for bb in range(2):
                        b = 2 * h + bb
                        nc.tensor.matmul(
                            ps[:, bb * 512:(bb + 1) * 512],
                            lhsT=ones8[32 * b:32 * b + 2, :],
                            rhs=esq8[32 * b:32 * b + 2, q, :],
                            start=False, stop=True,
                            tile_position=(32 * b, 0),
                        )
                    nc.scalar.copy(
                        out=scq[q][:, h * 1024:(h + 1) * 1024], in_=ps[:])

                # tree: pooled pair halves at [0:2048],[2048:4096] (pairs
                # (j, j+4096)); pooled2 (4-code slots) at [4096:6144];
                # L4 (8-code groups) at [6144:7168]
                pooled = plp.tile([P, 7168], F16)
                nc.vector.tensor_tensor(
                    out=pooled[:, 0:2048], in0=scq[0][:],
                    in1=scq[2][:], op=mybir.AluOpType.max,
                )
                nc.vector.tensor_tensor(
                    out=pooled[:, 2048:HK], in0=scq[1][:],
                    in1=scq[3][:], op=mybir.AluOpType.max,
                )
                nc.vector.tensor_tensor(
                    out=pooled[:, HK:HK + 2048], in0=pooled[:, 0:2048],
                    in1=pooled[:, 2048:HK], op=mybir.AluOpType.max,
                )
                nc.vector.tensor_tensor(
                    out=pooled[:, HK + 2048:HK + 3072],
                    in0=pooled[:, HK:HK + 1024],
                    in1=pooled[:, HK + 1024:HK + 2048],
                    op=mybir.AluOpType.max,
                )
                mx = small.tile([P, 8], F16, tag="mx")
                nc.vector.max(out=mx[:], in_=pooled[:, HK + 2048:HK + 3072])
                fi = small.tile([P, 8], U32, tag="fi")
                nc.vector.max_index(
                    out=fi[:], in_max=mx[:], in_values=pooled[:, HK:HK + 2048]
                )


                # two slot-gathers: row = [e_a, -esq_a, e_b, -esq_b, e_c,
                # -esq_c, e_d, -esq_d] covering codes {j, j+2048, j+4096,
                # j+6144}
                q2 = []
                for s in range(2):
                    qs = gat.tile([P, GW4], F32, tag=f"q{s}", name=f"q{s}")
                    nc.gpsimd.indirect_dma_start(
                        out=qs[:], out_offset=None, in_=tab4_d[:],
                        in_offset=IndirectOffsetOnAxis(
                            ap=fi[:, s:s + 1], axis=0),
                    )
                    q2.append(qs)
                state[ti] = (q2, fi, x2row)

            def tile_back(ti):
                tsl = slice(ti * P, (ti + 1) * P)
                q2, fi, x2row = state.pop(ti)

                # exact rescore of 8 candidates: per gathered tile, one big
                # multiply (x2 broadcast across the 4 code chunks) + one
                # segmented tensor_reduce + one contiguous FD4 add of the
                # gathered -|e|^2 tail. g0 on DVE, g1 on gpsimd.
                sc8 = small.tile([P, 8], F32, tag="sc8")
                x2b = x2row[:].rearrange("p (o d) -> p o d", o=1).to_broadcast([P, 4, DIM + 1])
                for g in range(2):
                    ms = msp.tile([P, 4, DIM + 1], F32, tag=f"ms{g}")
                    nc.gpsimd.tensor_tensor(
                        out=ms[:],
                        in0=q2[g][:].rearrange("p (c d) -> p c d", d=DIM + 1),
                        in1=x2b, op=mybir.AluOpType.mult,
                    )
                    nc.vector.tensor_reduce(
                        out=sc8[:, g * 4:(g + 1) * 4], in_=ms[:],
                        axis=mybir.AxisListType.X, op=mybir.AluOpType.add,
                    )

                # candidate ids: ids8[s] = slot_g + 2048*c  (int path)
                ids8 = small.tile([P, 8], I32, tag="ids8")
                nc.vector.tensor_tensor(
                    out=ids8[:].rearrange("p (g c) -> p g c", g=2),
                    in0=fi[:, 0:2].rearrange("p (g c) -> p g c", c=1).to_broadcast([P, 2, 4]),
                    in1=const8[:].rearrange("p (g c) -> p g c", g=2),
                    op=mybir.AluOpType.add,
                )
                m1 = small.tile([P, 1], F32, tag="m1")
                nc.vector.reduce_max(
                    out=m1[:], in_=sc8[:], axis=mybir.AxisListType.X
                )
                mask8 = small.tile([P, 8], I32, tag="mask8")
                nc.vector.tensor_scalar(
                    out=mask8[:], in0=sc8[:], scalar1=m1[:, 0:1], scalar2=None,
                    op0=mybir.AluOpType.is_ge,
                )
                mid8 = small.tile([P, 8], I32, tag="mid8")
                nc.vector.tensor_tensor(
                    out=mid8[:], in0=mask8[:], in1=ids8[:],
                    op=mybir.AluOpType.mult,
                )
                wid = small.tile([P, 1], I32, tag="wid")
                nc.vector.reduce_max(
                    out=wid[:], in_=mid8[:], axis=mybir.AxisListType.X
                )

                outw = gat.tile([P, DIM], F32, tag="outw", name="outw")
                nc.gpsimd.indirect_dma_start(
                    out=outw[:], out_offset=None, in_=tabW_d[:],
                    in_offset=IndirectOffsetOnAxis(ap=wid[:, 0:1], axis=0),
                )
                nc.gpsimd.dma_start(out=out_d[tsl, :], in_=outw[:])

            for ti in range(n_tt):
                tile_front(ti)
                if ti > 0:
                    tile_back(ti - 1)
            tile_back(n_tt - 1)

    nc.compile()
    return nc


def prep_core_inputs(x_i: np.ndarray, shared: dict) -> dict:
    x2 = (2.0 * x_i).astype(np.float32)
    xTh = np.ascontiguousarray(x2.astype(NF16).T)
    x2e = np.concatenate(
        [x2, np.ones((x2.shape[0], 1), np.float32)], axis=1)
    return {
        "xTh": xTh,
        "x2": np.ascontiguousarray(x2e),
        **shared,
    }


def prep_shared(embed: np.ndarray) -> dict:
    # permute codebook (evens then odds): pool pair j = permuted rows
    # (j, j+K/2); pooled2 slot j covers {j, j+2048, j+4096, j+6144}
    perm = np.concatenate([np.arange(0, K, 2), np.arange(1, K, 2)])
    eP = embed[perm]
    esq = (eP.astype(np.float64) ** 2).sum(1).astype(np.float32)
    neg = -esq
    hi = neg.astype(NF16)
    lo = (neg - hi.astype(np.float32)).astype(NF16)
    # esq8[b, r, q, j] = (hi,lo)[r][q*2048 + b*512 + j]
    esqr = np.stack([hi, lo], axis=0).reshape(2, NQ, 4, 512)  # [r, q, b, j]
    esq8 = np.ascontiguousarray(esqr.transpose(2, 0, 1, 3))   # [b, r, q, j]
    ebT = np.ascontiguousarray(eP.astype(NF16).T)
    # tab4 row j = [e_j, -esq_j, e_{j+2048}, -esq_{j+2048}, ...] so that a
    # 257-wide segmented reduce of q*x2e yields dot - |e|^2 directly
    blocks = []
    for c in range(4):
        sl = slice(c * QK, (c + 1) * QK)
        blocks += [eP[sl], neg[sl, None]]
    tab4 = np.concatenate(blocks, axis=1).astype(np.float32)
    return {
        "ebT": ebT,
        "esq8": esq8,
        "tab4": np.ascontiguousarray(tab4),
        "tabW": np.ascontiguousarray(eP.astype(np.float32)),
    }


def kernel(x: np.ndarray, embed: np.ndarray) -> np.ndarray:
    x = np.ascontiguousarray(x, dtype=np.float32)
    embed = np.ascontiguousarray(embed, dtype=np.float32)
    assert x.shape == (B, T, DIM), x.shape
    assert embed.shape == (K, DIM), embed.shape

    nc = build_nc(T)
    shared = prep_shared(embed)
    in_maps = [prep_core_inputs(x[i], shared) for i in range(N_CORES)]
    res = run_bass_kernel_spmd(nc, in_maps, core_ids=list(range(N_CORES)))
    out = np.stack([res.results[i]["out"] for i in range(N_CORES)], axis=0)
    return out.astype(np.float32)


if __name__ == "__main__":
    rng = np.random.default_rng(0)
    x = rng.standard_normal((B, T, DIM), dtype=np.float32)
    embed = rng.standard_normal((K, DIM), dtype=np.float32)
    out = kernel(x, embed)
    flat = x.reshape(-1, DIM)
    d = (flat * flat).sum(1)[:, None] - 2.0 * flat @ embed.T + (embed * embed).sum(1)[None, :]
    ref = embed[np.argmin(d, axis=1)].reshape(B, T, DIM)
    err = np.abs(out - ref).max()
    print("max abs err vs numpy ref:", err)


# revision 9
# speedup vs baseline: 1.2914x; 1.0085x over previous
"""VQ codebook kernel v5 for 8 TRN2 NeuronCores.

Data-parallel over batch: each core handles one batch element (4096 tokens).

Per 128-token tile:
  - PE (fp16): coarse scores s[t,k] = fp16(2x_t).fp16(e_k) - |e_k|^2 into
    PSUM quarters; the -|e|^2 rows (fp16 hi+lo) are added by four K=2
    matmuls per quarter packed into distinct 32-row groups via
    tile_position so they run concurrently (~1 matmul slot instead of 4).
  - ACT evicts each quarter to fp16 SBUF scores.
  - DVE tree: pooled pair halves (split so each starts as soon as its two
    quarters are evicted) -> pooled2[2048] (4-code slots) -> L4[1024]
    (8-code groups); max8 over L4; find_index8 over pooled2 locates the
    best 4-code slot of the top-2 groups (duplicate in_max values return
    distinct positions, covering f16 ties).
  - gather: tab4 rows hold all 4 codes of a slot (+ their -|e|^2); two
    indirect gathers fetch 8 candidate codes (descriptor-count-bound, so
    the wide rows cost the same as narrow ones).
  - Rescore: per gathered tile one big multiply (x2 broadcast over the 4
    code chunks) + one segmented tensor_reduce + one FD4 add of the
    gathered -|e|^2 tail; tile g0 on DVE, g1 on gpsimd. All exact f32.
  - Winner: integer id math (slot id + 2048*c), is_ge mask, masked-max
    reduce -> id; indirect gather from the f32 codebook + store.
"""

import numpy as np
import ml_dtypes

import concourse.bacc as bacc
import concourse.bass as bass
import concourse.mybir as mybir
from concourse.bass import IndirectOffsetOnAxis
from concourse.bass_utils import run_bass_kernel_spmd
from concourse.tile import TileContext

DIM = 256
K = 8192
B = 8
T = 4096
N_CORES = 8
P = 128
NQ = 4
QK = K // NQ          # 2048
HK = K // 2           # 4096
GW4 = 4 * (DIM + 1)   # 1028 floats per tab4 row
F32 = mybir.dt.float32
F16 = mybir.dt.float16
I32 = mybir.dt.int32
U32 = mybir.dt.uint32
NF16 = np.float16
N_GP_STT = 3          # candidates rescored on gpsimd (rest on DVE)


def build_nc(t_local: int = T) -> bass.Bass:
    assert t_local % P == 0
    n_tt = t_local // P

    nc = bacc.Bacc("TRN2", target_bir_lowering=False, debug=False)
    xTh_d = nc.declare_dram_parameter("xTh", [DIM, t_local], F16, isOutput=False)
    x2_d = nc.declare_dram_parameter("x2", [t_local, DIM + 1], F32, isOutput=False)
    ebT_d = nc.declare_dram_parameter("ebT", [DIM, K], F16, isOutput=False)
    esq8_d = nc.declare_dram_parameter("esq8", [4, 2, NQ, 512], F16, isOutput=False)
    tab4_d = nc.declare_dram_parameter("tab4", [QK, GW4], F32, isOutput=False)
    tabW_d = nc.declare_dram_parameter("tabW", [K, DIM], F32, isOutput=False)
    out_d = nc.declare_dram_parameter("out", [t_local, DIM], F32, isOutput=True)

    with TileContext(nc) as tc:
        with (
            tc.tile_pool(name="persist", bufs=1) as pp,
            tc.tile_pool(name="psum", bufs=4, space="PSUM") as psum_pool,
            tc.tile_pool(name="xload", bufs=4) as xload,
            tc.tile_pool(name="scores", bufs=3) as scp,
            tc.tile_pool(name="pool", bufs=2) as plp,
            tc.tile_pool(name="gat", bufs=3) as gat,
            tc.tile_pool(name="ms", bufs=4) as msp,
            tc.tile_pool(name="small", bufs=4) as small,
        ):
            ebT = pp.tile([P, 2, K], F16)
            nc.sync.dma_start(
                out=ebT[:], in_=ebT_d[:].rearrange("(a b) k -> b a k", a=2)
            )
            # -|e|^2 fp16 hi+lo rows replicated at partition bases 0/32/64/96
            # so the four aug matmuls of a quarter can row-tile concurrently
            esq8 = pp.tile([P, NQ, 512], F16)
            for b in range(4):
                nc.sync.dma_start(
                    out=esq8[32 * b:32 * b + 2, :, :], in_=esq8_d[b]
                )
            ones8 = pp.tile([P, P], F16)
            nc.vector.memset(ones8[:], 1.0)
            # candidate id offsets: [0,2048,4096,6144] per slot, both slots
            const8 = pp.tile([P, 8], I32)
            nc.gpsimd.iota(const8[:], pattern=[[0, 2], [2048, 4]], base=0,
                           channel_multiplier=0)

            state = {}

            def tile_front(ti):
                tsl = slice(ti * P, (ti + 1) * P)
                xThr = xload.tile([P, 2, P], F16, tag="xThr")
                nc.sync.dma_start(
                    out=xThr[:],
                    in_=xTh_d[:, tsl].rearrange("(a b) t -> b a t", a=2),
                )
                x2row = xload.tile([P, DIM + 1], F32, tag="x2row")
                nc.sync.dma_start(out=x2row[:], in_=x2_d[tsl, :])

                scq = [scp.tile([P, QK], F16, tag=f"scq{q}", name=f"scq{q}")
                       for q in range(NQ)]
                for e in range(8):
                    q, h = divmod(e, 2)
                    ps = psum_pool.tile([P, 1024], F32)
                    for c in range(2):
                        for bb in range(2):
                            b = 2 * h + bb
                            ksl = slice(q * QK + b * 512, q * QK + (b + 1) * 512)
                            nc.tensor.matmul(
                                ps[:, bb * 512:(bb + 1) * 512],
                                lhsT=xThr[:, c, :], rhs=ebT[:, c, ksl],
                                start=(c == 0), stop=False,
                            )
                    for bb in range(2):
                        b = 2 * h + bb
                        nc.tensor.matmul(
                            ps[:, bb * 512:(bb + 1) * 512],
                            lhsT=ones8[32 * b:32 * b + 2, :],
                            rhs=esq8[32 * b:32 * b + 2, q, :],
                            start=False, stop=True,
                            tile_position=(32 * b, 0),
                        )
                    nc.scalar.copy(
                        out=scq[q][:, h * 1024:(h + 1) * 1024], in_=ps[:])

                # tree: pooled pair halves at [0:2048],[2048:4096] (pairs
                # (j, j+4096)); pooled2 (4-code slots) at [4096:6144];
                # L4 (8-code groups) at [6144:7168]
                pooled = plp.tile([P, 7168], F16)
                nc.vector.tensor_tensor(
                    out=pooled[:, 0:2048], in0=scq[0][:],
                    in1=scq[2][:], op=mybir.AluOpType.max,
                )
                nc.vector.tensor_tensor(
                    out=pooled[:, 2048:HK], in0=scq[1][:],
                    in1=scq[3][:], op=mybir.AluOpType.max,
                )
                nc.vector.tensor_tensor(
                    out=pooled[:, HK:HK + 2048], in0=pooled[:, 0:2048],
                    in1=pooled[:, 2048:HK], op=mybir.AluOpType.max,
                )
                nc.vector.tensor_tensor(
                    out=pooled[:, HK + 2048:HK + 3072],
                    in0=pooled[:, HK:HK + 1024],
                    in1=pooled[:, HK + 1024:HK + 2048],
                    op=mybir.AluOpType.max,
                )
                mx = small.tile([P, 8], F16, tag="mx")
                nc.vector.max(out=mx[:], in_=pooled[:, HK + 2048:HK + 3072])
                fi = small.tile([P, 8], U32, tag="fi")
                nc.vector.max_index(
                    out=fi[:], in_max=mx[:], in_values=pooled[:, HK:HK + 2048]
                )


                # two slot-gathers: row = [e_a, -esq_a, e_b, -esq_b, e_c,
                # -esq_c, e_d, -esq_d] covering codes {j, j+2048, j+4096,
                # j+6144}
                q2 = []
                for s in range(2):
                    qs = gat.tile([P, GW4], F32, tag=f"q{s}", name=f"q{s}")
                    nc.gpsimd.indirect_dma_start(
                        out=qs[:], out_offset=None, in_=tab4_d[:],
                        in_offset=IndirectOffsetOnAxis(
                            ap=fi[:, s:s + 1], axis=0),
                    )
                    q2.append(qs)
                state[ti] = (q2, fi, x2row)

            def tile_back(ti):
                tsl = slice(ti * P, (ti + 1) * P)
                q2, fi, x2row = state.pop(ti)

                # exact rescore of 8 candidates: per gathered tile, one big
                # multiply (x2 broadcast across the 4 code chunks) + one
                # segmented tensor_reduce + one contiguous FD4 add of the
                # gathered -|e|^2 tail. g0 on DVE, g1 on gpsimd.
                sc8 = small.tile([P, 8], F32, tag="sc8")
                x2b = x2row[:].rearrange("p (o d) -> p o d", o=1).to_broadcast([P, 4, DIM + 1])
                for g in range(2):
                    ms = msp.tile([P, 4, DIM + 1], F32, tag=f"ms{g}")
                    nc.gpsimd.tensor_tensor(
                        out=ms[:],
                        in0=q2[g][:].rearrange("p (c d) -> p c d", d=DIM + 1),
                        in1=x2b, op=mybir.AluOpType.mult,
                    )
                    nc.vector.tensor_reduce(
                        out=sc8[:, g * 4:(g + 1) * 4], in_=ms[:],
                        axis=mybir.AxisListType.X, op=mybir.AluOpType.add,
                    )

                # candidate ids: ids8[s] = slot_g + 2048*c  (int path)
                ids8 = small.tile([P, 8], I32, tag="ids8")
                nc.vector.tensor_tensor(
                    out=ids8[:].rearrange("p (g c) -> p g c", g=2),
                    in0=fi[:, 0:2].rearrange("p (g c) -> p g c", c=1).to_broadcast([P, 2, 4]),
                    in1=const8[:].rearrange("p (g c) -> p g c", g=2),
                    op=mybir.AluOpType.add,
                )
                m1 = small.tile([P, 1], F32, tag="m1")
                nc.vector.reduce_max(
                    out=m1[:], in_=sc8[:], axis=mybir.AxisListType.X
                )
                mask8 = small.tile([P, 8], I32, tag="mask8")
                nc.vector.tensor_scalar(
                    out=mask8[:], in0=sc8[:], scalar1=m1[:, 0:1], scalar2=None,
                    op0=mybir.AluOpType.is_ge,
                )
                mid8 = small.tile([P, 8], I32, tag="mid8")
                nc.vector.tensor_tensor(
                    out=mid8[:], in0=mask8[:], in1=ids8[:],
                    op=mybir.AluOpType.mult,
                )
                wid = small.tile([P, 1], I32, tag="wid")
                nc.vector.reduce_max(
                    out=wid[:], in_=mid8[:], axis=mybir.AxisListType.X
                )

                outw = gat.tile([P, DIM], F32, tag="outw", name="outw")
                nc.gpsimd.indirect_dma_start(
                    out=outw[:], out_offset=None, in_=tabW_d[:],
                    in_offset=IndirectOffsetOnAxis(ap=wid[:, 0:1], axis=0),
                )
                nc.sync.dma_start(out=out_d[tsl, :], in_=outw[:])

            for ti in range(n_tt):
                if ti > 0:
                    tile_back(ti - 1)
                tile_front(ti)
            tile_back(n_tt - 1)

    nc.compile()
    return nc


def prep_core_inputs(x_i: np.ndarray, shared: dict) -> dict:
    x2 = (2.0 * x_i).astype(np.float32)
    xTh = np.ascontiguousarray(x2.astype(NF16).T)
    x2e = np.concatenate(
        [x2, np.ones((x2.shape[0], 1), np.float32)], axis=1)
    return {
        "xTh": xTh,
        "x2": np.ascontiguousarray(x2e),
        **shared,
    }


def prep_shared(embed: np.ndarray) -> dict:
    # permute codebook (evens then odds): pool pair j = permuted rows
    # (j, j+K/2); pooled2 slot j covers {j, j+2048, j+4096, j+6144}
    perm = np.concatenate([np.arange(0, K, 2), np.arange(1, K, 2)])
    eP = embed[perm]
    esq = (eP.astype(np.float64) ** 2).sum(1).astype(np.float32)
    neg = -esq
    hi = neg.astype(NF16)
    lo = (neg - hi.astype(np.float32)).astype(NF16)
    # esq8[b, r, q, j] = (hi,lo)[r][q*2048 + b*512 + j]
    esqr = np.stack([hi, lo], axis=0).reshape(2, NQ, 4, 512)  # [r, q, b, j]
    esq8 = np.ascontiguousarray(esqr.transpose(2, 0, 1, 3))   # [b, r, q, j]
    ebT = np.ascontiguousarray(eP.astype(NF16).T)
    # tab4 row j = [e_j, -esq_j, e_{j+2048}, -esq_{j+2048}, ...] so that a
    # 257-wide segmented reduce of q*x2e yields dot - |e|^2 directly
    blocks = []
    for c in range(4):
        sl = slice(c * QK, (c + 1) * QK)
        blocks += [eP[sl], neg[sl, None]]
    tab4 = np.concatenate(blocks, axis=1).astype(np.float32)
    return {
        "ebT": ebT,
        "esq8": esq8,
        "tab4": np.ascontiguousarray(tab4),
        "tabW": np.ascontiguousarray(eP.astype(np.float32)),
    }


def kernel(x: np.ndarray, embed: np.ndarray) -> np.ndarray:
    x = np.ascontiguousarray(x, dtype=np.float32)
    embed = np.ascontiguousarray(embed, dtype=np.float32)
    assert x.shape == (B, T, DIM), x.shape
    assert embed.shape == (K, DIM), embed.shape

    nc = build_nc(T)
    shared = prep_shared(embed)
    in_maps = [prep_core_inputs(x[i], shared) for i in range(N_CORES)]
    res = run_bass_kernel_spmd(nc, in_maps, core_ids=list(range(N_CORES)))
    out = np.stack([res.results[i]["out"] for i in range(N_CORES)], axis=0)
    return out.astype(np.float32)


if __name__ == "__main__":
    rng = np.random.default_rng(0)
    x = rng.standard_normal((B, T, DIM), dtype=np.float32)
    embed = rng.standard_normal((K, DIM), dtype=np.float32)
    out = kernel(x, embed)
    flat = x.reshape(-1, DIM)
    d = (flat * flat).sum(1)[:, None] - 2.0 * flat @ embed.T + (embed * embed).sum(1)[None, :]
    ref = embed[np.argmin(d, axis=1)].reshape(B, T, DIM)
    err = np.abs(out - ref).max()
    print("max abs err vs numpy ref:", err)


# revision 11
# speedup vs baseline: 1.3575x; 1.0511x over previous
"""VQ codebook kernel v7 for 8 TRN2 NeuronCores.

Data-parallel over batch: each core handles one batch element (4096 tokens).

Per 128-token tile (software-pipelined: the rescore/emit of tile t-1 is
emitted after the scores/scan of tile t):
  - PE (fp16): coarse scores s[t,k] = fp16(2x_t).fp16(e_k) - |e_k|^2 in
    eighth-of-K PSUM chunks ([128,1024] x 8, 4-deep rotation) so the PE can
    run ahead of evictions; the -|e|^2 rows (fp16 hi+lo) are added by K=2
    matmuls packed into distinct 32-row groups via tile_position (they
    execute concurrently, ~4ns apart).
  - ACT evicts each eighth to fp16 quarter-score tiles (per-quarter tiles
    so each eviction is gated only by its own tree reader).
  - DVE tree: TT-max(q0,q2) and TT-max(q1,q3) build pair-pooled halves
    (pairs (j, j+4096) via host permutation), then pooled2[2048] (4-code
    slots) and L4[1024] (8-code groups); max8 over L4; find_index8 over
    pooled2 locates the best 4-code slot of the top-2 groups (duplicate
    in_max values return distinct positions, covering f16 ties; verified
    offline: 0/32768 wrong rows).
  - gather: tab4 rows interleave [e_c, -esq_c] x 4 for the slot's codes;
    two indirect gathers fetch 8 candidates (indirect DMA is
    descriptor-count-bound, so 4KB rows cost the same as 1KB rows).
  - Rescore: per gathered tile, one gpsimd multiply against x2e (x2 with a
    trailing ones column, broadcast across the 4 chunks) and one DVE
    257-wide segmented tensor_reduce whose segment sums are directly
    dot - |e|^2 (exact f32; the ones column times -esq folds the bias in).
  - Winner: integer ids (slot + 2048*c via iota const), is_ge mask,
    masked-max reduce -> id; indirect gather from the f32 codebook + store.
"""

import numpy as np
import ml_dtypes

import concourse.bacc as bacc
import concourse.bass as bass
import concourse.mybir as mybir
from concourse.bass import IndirectOffsetOnAxis
from concourse.bass_utils import run_bass_kernel_spmd
from concourse.tile import TileContext

DIM = 256
K = 8192
B = 8
T = 4096
N_CORES = 8
P = 128
NQ = 4
QK = K // NQ          # 2048
HK = K // 2           # 4096
GW4 = 4 * (DIM + 1)   # 1028 floats per tab4 row
F32 = mybir.dt.float32
F16 = mybir.dt.float16
I32 = mybir.dt.int32
U32 = mybir.dt.uint32
NF16 = np.float16
N_GP_STT = 3          # candidates rescored on gpsimd (rest on DVE)


def build_nc(t_local: int = T) -> bass.Bass:
    assert t_local % P == 0
    n_tt = t_local // P

    nc = bacc.Bacc("TRN2", target_bir_lowering=False, debug=False)
    xTh_d = nc.declare_dram_parameter("xTh", [DIM, t_local], F16, isOutput=False)
    x2_d = nc.declare_dram_parameter("x2", [t_local, DIM + 1], F32, isOutput=False)
    ebT_d = nc.declare_dram_parameter("ebT", [DIM, K], F16, isOutput=False)
    esq8_d = nc.declare_dram_parameter("esq8", [4, 2, NQ, 512], F16, isOutput=False)
    tab4_d = nc.declare_dram_parameter("tab4", [QK, GW4], F32, isOutput=False)
    tabW_d = nc.declare_dram_parameter("tabW", [K, DIM], F32, isOutput=False)
    out_d = nc.declare_dram_parameter("out", [t_local, DIM], F32, isOutput=True)

    with TileContext(nc) as tc:
        with (
            tc.tile_pool(name="persist", bufs=1) as pp,
            tc.tile_pool(name="psum", bufs=4, space="PSUM") as psum_pool,
            tc.tile_pool(name="xload", bufs=4) as xload,
            tc.tile_pool(name="scores", bufs=3) as scp,
            tc.tile_pool(name="pool", bufs=2) as plp,
            tc.tile_pool(name="gat", bufs=3) as gat,
            tc.tile_pool(name="ms", bufs=4) as msp,
            tc.tile_pool(name="small", bufs=4) as small,
        ):
            ebT = pp.tile([P, 2, K], F16)
            nc.sync.dma_start(
                out=ebT[:], in_=ebT_d[:].rearrange("(a b) k -> b a k", a=2)
            )
            # -|e|^2 fp16 hi+lo rows replicated at partition bases 0/32/64/96
            # so the four aug matmuls of a quarter can row-tile concurrently
            esq8 = pp.tile([P, NQ, 512], F16)
            for b in range(4):
                nc.sync.dma_start(
                    out=esq8[32 * b:32 * b + 2, :, :], in_=esq8_d[b]
                )
            ones8 = pp.tile([P, P], F16)
            nc.vector.memset(ones8[:], 1.0)
            # candidate id offsets: [0,2048,4096,6144] per slot, both slots
            const8 = pp.tile([P, 8], I32)
            nc.gpsimd.iota(const8[:], pattern=[[0, 2], [2048, 4]], base=0,
                           channel_multiplier=0)

            state = {}

            def tile_front(ti):
                tsl = slice(ti * P, (ti + 1) * P)
                xThr = xload.tile([P, 2, P], F16, tag="xThr")
                nc.sync.dma_start(
                    out=xThr[:],
                    in_=xTh_d[:, tsl].rearrange("(a b) t -> b a t", a=2),
                )
                x2row = xload.tile([P, DIM + 1], F32, tag="x2row")
                nc.sync.dma_start(out=x2row[:], in_=x2_d[tsl, :])

                scq = [scp.tile([P, QK], F16, tag=f"scq{q}", name=f"scq{q}")
                       for q in range(NQ)]
                for e in range(8):
                    q, h = divmod(e, 2)
                    ps = psum_pool.tile([P, 1024], F32)
                    for c in range(2):
                        for bb in range(2):
                            b = 2 * h + bb
                            ksl = slice(q * QK + b * 512, q * QK + (b + 1) * 512)
                            nc.tensor.matmul(
                                ps[:, bb * 512:(bb + 1) * 512],
                                lhsT=xThr[:, c, :], rhs=ebT[:, c, ksl],
                                start=(c == 0), stop=False,
                            )
                    for bb in range(2):
                        b = 2 * h + bb
                        nc.tensor.matmul(
                            ps[:, bb * 512:(bb + 1) * 512],
                            lhsT=ones8[32 * b:32 * b + 2, :],
                            rhs=esq8[32 * b:32 * b + 2, q, :],
                            start=False, stop=True,
                            tile_position=(32 * b, 0),
                        )
                    nc.scalar.copy(
                        out=scq[q][:, h * 1024:(h + 1) * 1024], in_=ps[:])

                # tree: pooled pair halves at [0:2048],[2048:4096] (pairs
                # (j, j+4096)); pooled2 (4-code slots) at [4096:6144];
                # L4 (8-code groups) at [6144:7168]
                pooled = plp.tile([P, 7168], F16)
                nc.vector.tensor_tensor(
                    out=pooled[:, 0:2048], in0=scq[0][:],
                    in1=scq[2][:], op=mybir.AluOpType.max,
                )
                nc.vector.tensor_tensor(
                    out=pooled[:, 2048:HK], in0=scq[1][:],
                    in1=scq[3][:], op=mybir.AluOpType.max,
                )
                nc.vector.tensor_tensor(
                    out=pooled[:, HK:HK + 2048], in0=pooled[:, 0:2048],
                    in1=pooled[:, 2048:HK], op=mybir.AluOpType.max,
                )
                nc.vector.tensor_tensor(
                    out=pooled[:, HK + 2048:HK + 3072],
                    in0=pooled[:, HK:HK + 1024],
                    in1=pooled[:, HK + 1024:HK + 2048],
                    op=mybir.AluOpType.max,
                )
                mx = small.tile([P, 8], F16, tag="mx")
                nc.vector.max(out=mx[:], in_=pooled[:, HK + 2048:HK + 3072])
                fi = small.tile([P, 8], U32, tag="fi")
                nc.vector.max_index(
                    out=fi[:], in_max=mx[:], in_values=pooled[:, HK:HK + 2048]
                )


                # two slot-gathers: row = [e_a, -esq_a, e_b, -esq_b, e_c,
                # -esq_c, e_d, -esq_d] covering codes {j, j+2048, j+4096,
                # j+6144}
                q2 = []
                for s in range(2):
                    qs = gat.tile([P, GW4], F32, tag=f"q{s}", name=f"q{s}")
                    nc.gpsimd.indirect_dma_start(
                        out=qs[:], out_offset=None, in_=tab4_d[:],
                        in_offset=IndirectOffsetOnAxis(
                            ap=fi[:, s:s + 1], axis=0),
                    )
                    q2.append(qs)
                state[ti] = (q2, fi, x2row)

            def tile_back(ti):
                tsl = slice(ti * P, (ti + 1) * P)
                q2, fi, x2row = state.pop(ti)

                # exact rescore of 8 candidates: per gathered tile, one big
                # multiply (x2 broadcast across the 4 code chunks) + one
                # segmented tensor_reduce + one contiguous FD4 add of the
                # gathered -|e|^2 tail. g0 on DVE, g1 on gpsimd.
                sc8 = small.tile([P, 8], F32, tag="sc8")
                x2b = x2row[:].rearrange("p (o d) -> p o d", o=1).to_broadcast([P, 4, DIM + 1])
                for g in range(2):
                    ms = msp.tile([P, 4, DIM + 1], F32, tag=f"ms{g}")
                    nc.gpsimd.tensor_tensor(
                        out=ms[:],
                        in0=q2[g][:].rearrange("p (c d) -> p c d", d=DIM + 1),
                        in1=x2b, op=mybir.AluOpType.mult,
                    )
                    nc.vector.tensor_reduce(
                        out=sc8[:, g * 4:(g + 1) * 4], in_=ms[:],
                        axis=mybir.AxisListType.X, op=mybir.AluOpType.add,
                    )

                # candidate ids: ids8[s] = slot_g + 2048*c  (int path)
                ids8 = small.tile([P, 8], I32, tag="ids8")
                nc.vector.tensor_tensor(
                    out=ids8[:].rearrange("p (g c) -> p g c", g=2),
                    in0=fi[:, 0:2].rearrange("p (g c) -> p g c", c=1).to_broadcast([P, 2, 4]),
                    in1=const8[:].rearrange("p (g c) -> p g c", g=2),
                    op=mybir.AluOpType.add,
                )
                m1 = small.tile([P, 1], F32, tag="m1")
                nc.vector.reduce_max(
                    out=m1[:], in_=sc8[:], axis=mybir.AxisListType.X
                )
                mask8 = small.tile([P, 8], I32, tag="mask8")
                nc.vector.tensor_scalar(
                    out=mask8[:], in0=sc8[:], scalar1=m1[:, 0:1], scalar2=None,
                    op0=mybir.AluOpType.is_ge,
                )
                mid8 = small.tile([P, 8], I32, tag="mid8")
                nc.vector.tensor_tensor(
                    out=mid8[:], in0=mask8[:], in1=ids8[:],
                    op=mybir.AluOpType.mult,
                )
                wid = small.tile([P, 1], I32, tag="wid")
                nc.vector.reduce_max(
                    out=wid[:], in_=mid8[:], axis=mybir.AxisListType.X
                )

                outw = gat.tile([P, DIM], F32, tag="outw", name="outw")
                nc.gpsimd.indirect_dma_start(
                    out=outw[:], out_offset=None, in_=tabW_d[:],
                    in_offset=IndirectOffsetOnAxis(ap=wid[:, 0:1], axis=0),
                )
                nc.gpsimd.dma_start(out=out_d[tsl, :], in_=outw[:])

            for ti in range(n_tt):
                tile_front(ti)
                if ti > 0:
                    tile_back(ti - 1)
            tile_back(n_tt - 1)

    nc.compile()
    return nc


def prep_core_inputs(x_i: np.ndarray, shared: dict) -> dict:
    x2 = (2.0 * x_i).astype(np.float32)
    xTh = np.ascontiguousarray(x2.astype(NF16).T)
    x2e = np.concatenate(
        [x2, np.ones((x2.shape[0], 1), np.float32)], axis=1)
    return {
        "xTh": xTh,
        "x2": np.ascontiguousarray(x2e),
        **shared,
    }


def prep_shared(embed: np.ndarray) -> dict:
    # permute codebook (evens then odds): pool pair j = permuted rows
    # (j, j+K/2); pooled2 slot j covers {j, j+2048, j+4096, j+6144}
    perm = np.concatenate([np.arange(0, K, 2), np.arange(1, K, 2)])
    eP = embed[perm]
    esq = (eP.astype(np.float64) ** 2).sum(1).astype(np.float32)
    neg = -esq
    hi = neg.astype(NF16)
    lo = (neg - hi.astype(np.float32)).astype(NF16)
    # esq8[b, r, q, j] = (hi,lo)[r][q*2048 + b*512 + j]
    esqr = np.stack([hi, lo], axis=0).reshape(2, NQ, 4, 512)  # [r, q, b, j]
    esq8 = np.ascontiguousarray(esqr.transpose(2, 0, 1, 3))   # [b, r, q, j]
    ebT = np.ascontiguousarray(eP.astype(NF16).T)
    # tab4 row j = [e_j, -esq_j, e_{j+2048}, -esq_{j+2048}, ...] so that a
    # 257-wide segmented reduce of q*x2e yields dot - |e|^2 directly
    blocks = []
    for c in range(4):
        sl = slice(c * QK, (c + 1) * QK)
        blocks += [eP[sl], neg[sl, None]]
    tab4 = np.concatenate(blocks, axis=1).astype(np.float32)
    return {
        "ebT": ebT,
        "esq8": esq8,
        "tab4": np.ascontiguousarray(tab4),
        "tabW": np.ascontiguousarray(eP.astype(np.float32)),
    }


def kernel(x: np.ndarray, embed: np.ndarray) -> np.ndarray:
    x = np.ascontiguousarray(x, dtype=np.float32)
    embed = np.ascontiguousarray(embed, dtype=np.float32)
    assert x.shape == (B, T, DIM), x.shape
    assert embed.shape == (K, DIM), embed.shape

    nc = build_nc(T)
    shared = prep_shared(embed)
    in_maps = [prep_core_inputs(x[i], shared) for i in range(N_CORES)]
    res = run_bass_kernel_spmd(nc, in_maps, core_ids=list(range(N_CORES)))
    out = np.stack([res.results[i]["out"] for i in range(N_CORES)], axis=0)
    return out.astype(np.float32)


if __name__ == "__main__":
    rng = np.random.default_rng(0)
    x = rng.standard_normal((B, T, DIM), dtype=np.float32)
    embed = rng.standard_normal((K, DIM), dtype=np.float32)
    out = kernel(x, embed)
    flat = x.reshape(-1, DIM)
    d = (flat * flat).sum(1)[:, None] - 2.0 * flat @ embed.T + (embed * embed).sum(1)[None, :]
    ref = embed[np.argmin(d, axis=1)].reshape(B, T, DIM)
    err = np.abs(out - ref).max()
    print("max abs err vs numpy ref:", err)
